# revision 1
# baseline (speedup 1.0000x reference)
"""Trainium2 Bass kernel for nn_ContextEncoder (banded local attention encoder).

Reference computation (B=2, T=2048, D=512, H=8, dh=64, band half-width 32):
  xn   = LayerNorm(x) * g + b
  q    = ((xn @ Wp.T + bp) @ Wq.T + bq) / sqrt(dh)      per-head [B,T,H,dh]
  k, v = xn @ Wk.T + bk, xn @ Wv.T + bv
  s    = banded scores  (|i-j| <= 32), softmax over window
  ctx  = (a @ v_window) @ Wo.T + bo
  gate = sigmoid([x, ctx] @ Wg.T + bg)
  out  = x * (1 - gate) + ctx * gate

Sharding: sequence-parallel, 8 cores = 2 batches x 4 chunks of 512 tokens.
Each core gets its 512-token chunk plus a 32-token halo on each side
(zero-padded at sequence edges; per-core masks kill invalid positions),
computes its 512 output rows fully independently (no collectives), and the
host concatenates.

Device layout choices:
  - LayerNorm token-major (bn_stats), then DMA-xbar-transpose to feature-major
    xnT [d, tok] (bf16) which feeds every projection without further
    transposes.
  - qT/kT feature-major via weight-stationary matmuls; v token-major with a
    ones-column interleaved per head (so the attention A @ [1|V] matmul
    emits softmax denominators for free).
  - Scores computed transposed S^T[w, q] per (128-query block, head) over a
    192-wide key window; exp on ACT, multiplicative band/boundary mask on DVE
    (fp32 probabilities), A-stationary AV matmul -> ctx token-major,
    normalized by the reciprocal denominator (per-partition scalar).
  - ctx DMA-transposed to feature-major for the O/gate projections
    (token-major outputs), sigmoid on ACT, final blend elementwise.

All weights are pre-transposed/fused on host (LN gain folded into Wp/Wk/Wv,
1/sqrt(dh) folded into Wq) and shipped bf16.
"""

import numpy as np
import ml_dtypes

B, T, D = 2, 2048, 512
H, DH = 8, 64
WCTX = 32
NCORES = 8
CHUNK = 512          # tokens per core
NBLK = CHUNK // 128  # 4 query blocks per core
HALO = CHUNK + 2 * WCTX   # 576 tokens incl. halo
XROWS = 640          # x dram rows: 512 central + 32 left + 32 right + 64 pad
BF16 = ml_dtypes.bfloat16

_CACHE = {}


def _build_program(flags, stage=6, s4=5):
    """Builds the single-core Bass/Tile program (shared SPMD across 8 cores).

    flags: (bo_nonzero, bg_nonzero) -> emit optional bias adds.
    """
    import concourse.bass as bass
    import concourse.tile as tile
    import concourse.mybir as mybir
    from concourse import bacc

    f32 = mybir.dt.float32
    bf16 = mybir.dt.bfloat16
    AF = mybir.ActivationFunctionType
    ALU = mybir.AluOpType
    bo_nz, bg_nz = flags

    nc = bacc.Bacc(
        "TRN2",
        target_bir_lowering=False,
        debug=False,
        enable_asserts=False,
        num_devices=NCORES,
    )

    x_in = nc.dram_tensor("x", [XROWS, D], f32, kind="ExternalInput")
    xt_in = nc.dram_tensor("xt", [D, CHUNK], bf16, kind="ExternalInput")
    mA_in = nc.dram_tensor("mA", [128, NBLK, 128], bf16, kind="ExternalInput")
    mB_in = nc.dram_tensor("mB", [64, NBLK, 128], bf16, kind="ExternalInput")
    w_in = {
        n: nc.dram_tensor(n, [D, D], bf16, kind="ExternalInput")
        for n in ["wp", "wq", "wk", "wv", "wo", "wg1", "wg2"]
    }
    # feature-major per-partition biases, [128, 4] layout (col = d-tile)
    bql_in = nc.dram_tensor("bql", [128, 4], f32, kind="ExternalInput")  # bp_eff
    bqh_in = nc.dram_tensor("bqh", [128, 4], f32, kind="ExternalInput")  # bq/8
    bkl_in = nc.dram_tensor("bkl", [128, 4], f32, kind="ExternalInput")  # bk_eff
    bv_in = nc.dram_tensor("bv", [1, D], f32, kind="ExternalInput")      # bv_eff
    bo_in = nc.dram_tensor("bo", [1, D], f32, kind="ExternalInput")
    bg_in = nc.dram_tensor("bg", [1, D], f32, kind="ExternalInput")
    out_t = nc.dram_tensor("out", [CHUNK, D], f32, kind="ExternalOutput")

    def rep_ap(ap, axis_pos, n):
        """Insert a stride-0 dim of size n into an AP at free-dim position."""
        aps = [list(p) for p in ap.ap]
        aps.insert(axis_pos, [0, n])
        return bass.AP(tensor=ap.tensor, offset=ap.offset, ap=aps)

    with tile.TileContext(nc) as tc:
        with (
            tc.tile_pool(name="wpool", bufs=1) as wpool,
            tc.tile_pool(name="apool", bufs=1) as apool,
            tc.tile_pool(name="small", bufs=1) as small,
            tc.tile_pool(name="stats", bufs=6) as stats_pool,
            tc.tile_pool(name="attn", bufs=2) as attn_pool,
            tc.tile_pool(name="fin", bufs=3) as fin_pool,
            tc.tile_pool(name="pp", bufs=3, space="PSUM") as pp,
            tc.tile_pool(name="sp0", bufs=2, space="PSUM") as sp0,
            tc.tile_pool(name="sp1", bufs=1, space="PSUM") as sp1,
            tc.tile_pool(name="cp", bufs=2, space="PSUM") as cp,
        ):
            # ---- persistent SBUF tensors ----
            x_sb = apool.tile([128, 5, D], f32, tag="x")
            xn0 = apool.tile([128, 5, D], bf16, tag="xn0")
            xnT = apool.tile([128, 4, HALO], bf16, tag="xnT")
            qinT = apool.tile([128, 4, CHUNK], bf16, tag="qinT")
            qT = apool.tile([128, 4, CHUNK], bf16, tag="qT")
            kT = apool.tile([128, 4, HALO], bf16, tag="kT")
            v_sb = apool.tile([128, 5, 8 * 65], bf16, tag="v")
            xt_sb = apool.tile([128, 4, CHUNK], bf16, tag="xt")
            mA_sb = apool.tile([128, NBLK, 128], bf16, tag="mA")
            mB_sb = apool.tile([64, NBLK, 128], bf16, tag="mB")

            ws = {n: wpool.tile([128, 4, D], bf16, tag=n, name=n) for n in w_in}
            bql = small.tile([128, 4], f32, tag="bql")
            bqh = small.tile([128, 4], f32, tag="bqh")
            bkl = small.tile([128, 4], f32, tag="bkl")
            bv_bc = small.tile([128, D], f32, tag="bv_bc")
            eps_t = small.tile([128, 1], f32, tag="eps")

            # ---- input DMAs ----
            # x split per 128-token tile so LayerNorm starts after the first
            x_view = x_in[:].rearrange("(c p) d -> p c d", p=128)
            for t in range(5):
                nc.sync.dma_start(out=x_sb[:, t, :], in_=x_view[:, t, :])
            for n in w_in:
                nc.sync.dma_start(
                    out=ws[n][:], in_=w_in[n][:].rearrange("(c p) d -> p c d", p=128)
                )
            nc.sync.dma_start(
                out=xt_sb[:], in_=xt_in[:].rearrange("(c p) d -> p c d", p=128)
            )
            nc.sync.dma_start(out=mA_sb[:], in_=mA_in[:])
            nc.sync.dma_start(out=mB_sb[:], in_=mB_in[:])
            nc.sync.dma_start(out=bql[:], in_=bql_in[:])
            nc.sync.dma_start(out=bqh[:], in_=bqh_in[:])
            nc.sync.dma_start(out=bkl[:], in_=bkl_in[:])
            nc.gpsimd.dma_start(out=bv_bc[:], in_=bv_in[:].to_broadcast([128, D]))
            bo_bc = bg_bc = None
            if bo_nz:
                bo_bc = small.tile([128, D], f32, tag="bo_bc")
                nc.gpsimd.dma_start(out=bo_bc[:], in_=bo_in[:].to_broadcast([128, D]))
            if bg_nz:
                bg_bc = small.tile([128, D], f32, tag="bg_bc")
                nc.gpsimd.dma_start(out=bg_bc[:], in_=bg_in[:].to_broadcast([128, D]))

            ident = small.tile([128, 128], bf16, tag="ident")
            from concourse.masks import make_identity
            make_identity(nc, ident[:])

            nc.vector.memset(eps_t[:], 1e-5)
            # ones column per head in v (ones at interleaved position 65h)
            v_view = v_sb[:].rearrange("p t (h c) -> p t h c", c=65)
            nc.gpsimd.memset(v_view[:, :, :, 0:1], 1.0)

            # ---- stage 1: LayerNorm (token-major, permuted layout) ----
            for t in range(5):
                rows = 128 if t < 4 else 64
                st = stats_pool.tile([128, 6], f32, tag="st")
                mv = stats_pool.tile([128, 2], f32, tag="mv")
                rstd = stats_pool.tile([128, 1], f32, tag="rstd")
                nc.vector.bn_stats(out=st[:rows], in_=x_sb[:rows, t, :])
                nc.vector.bn_aggr(out=mv[:rows], in_=st[:rows])
                nc.scalar.activation(
                    out=rstd[:rows], in_=mv[:rows, 1:2], func=AF.Sqrt,
                    bias=eps_t[:rows], scale=1.0,
                )
                nc.vector.reciprocal(out=rstd[:rows], in_=rstd[:rows])
                nc.vector.tensor_scalar(
                    out=xn0[:rows, t, :], in0=x_sb[:rows, t, :],
                    scalar1=mv[:rows, 0:1], scalar2=rstd[:rows],
                    op0=ALU.subtract, op1=ALU.mult,
                )

            if stage >= 2:
                # ---- stage 2: transpose xn0 -> xnT via PE (halo-frame order) ----
                # x rows: [0:512] central (halo 32..544), [512:544] left halo
                # (halo 0..32), [544:576] right halo (halo 544..576)
                for j in range(4):
                    tp = pp.tile([128, 512], bf16, tag="pp", name=f"tpx{j}")
                    for t in range(4):
                        nc.tensor.transpose(
                            tp[:, 128 * t: 128 * (t + 1)],
                            xn0[:, t, 128 * j: 128 * (j + 1)],
                            ident[:],
                        )
                    nc.scalar.activation(
                        out=xnT[:, j, 32:544], in_=tp[:], func=AF.Copy
                    )
                    th = pp.tile([128, 512], bf16, tag="pp", name=f"thx{j}")
                    nc.tensor.transpose(
                        th[:, 0:64],
                        xn0[0:64, 4, 128 * j: 128 * (j + 1)],
                        ident[0:64, 0:64],
                    )
                    _base = xnT[:, j, :]
                    halo_out = bass.AP(
                        tensor=_base.tensor,
                        offset=_base.offset,
                        ap=[list(_base.ap[0]), [544, 2], [1, 32]],
                    )
                    nc.scalar.activation(
                        out=halo_out, in_=th[:, 0:64].rearrange("p (a b) -> p a b", b=32),
                        func=AF.Copy,
                    )

            if stage >= 3:
                # ---- stage 3: projections ----
                # qinT[d, q] = Wp_eff @ xnT  (central tokens only)
                for j in range(4):
                    ps = pp.tile([128, 512], f32, tag="pp")
                    for c in range(4):
                        nc.tensor.matmul(
                            ps[:], ws["wp"][:, c, 128 * j: 128 * (j + 1)],
                            xnT[:, c, 32: 32 + CHUNK],
                            start=(c == 0), stop=(c == 3),
                        )
                    nc.scalar.activation(
                        out=qinT[:, j, :], in_=ps[:], func=AF.Identity,
                        bias=bql[:, j: j + 1], scale=1.0,
                    )
                # qT[d, q] = (Wq/8) @ qinT
                for j in range(4):
                    ps = pp.tile([128, 512], f32, tag="pp")
                    for c in range(4):
                        nc.tensor.matmul(
                            ps[:], ws["wq"][:, c, 128 * j: 128 * (j + 1)],
                            qinT[:, c, :],
                            start=(c == 0), stop=(c == 3),
                        )
                    nc.scalar.activation(
                        out=qT[:, j, :], in_=ps[:], func=AF.Identity,
                        bias=bqh[:, j: j + 1], scale=1.0,
                    )
                # kT[d, w] = Wk_eff @ xnT  (all 576 halo tokens, split 512+64)
                for j in range(4):
                    ps = pp.tile([128, 512], f32, tag="pp")
                    ps2 = pp.tile([128, 512], f32, tag="pp")
                    for c in range(4):
                        nc.tensor.matmul(
                            ps[:], ws["wk"][:, c, 128 * j: 128 * (j + 1)],
                            xnT[:, c, 0:512],
                            start=(c == 0), stop=(c == 3),
                        )
                    for c in range(4):
                        nc.tensor.matmul(
                            ps2[:, 0:64], ws["wk"][:, c, 128 * j: 128 * (j + 1)],
                            xnT[:, c, 512:576],
                            start=(c == 0), stop=(c == 3),
                        )
                    nc.scalar.activation(
                        out=kT[:, j, 0:512], in_=ps[:], func=AF.Identity,
                        bias=bkl[:, j: j + 1], scale=1.0,
                    )
                    nc.scalar.activation(
                        out=kT[:, j, 512:576], in_=ps2[:, 0:64], func=AF.Identity,
                        bias=bkl[:, j: j + 1], scale=1.0,
                    )
                # v[w, d] token-major (+ bias broadcast), interleaved ones cols
                for t in range(5):
                    rows = 128 if t < 4 else 64
                    ps = pp.tile([128, 512], f32, tag="pp")
                    for c in range(4):
                        nc.tensor.matmul(
                            ps[:rows], xnT[:, c, 128 * t: 128 * t + rows],
                            ws["wv"][:, c, :],
                            start=(c == 0), stop=(c == 3),
                        )
                    nc.vector.tensor_add(
                        out=v_view[:rows, t, :, 1:65],
                        in0=ps[:rows].rearrange("p (h c) -> p h c", c=64),
                        in1=bv_bc[:rows].rearrange("p (h c) -> p h c", c=64),
                    )

            if stage >= 4:
                # gate part 1 (x @ Wg1.T) is independent of attention; compute
                # it early so only the ctx part remains on the critical tail
                g1_sb = apool.tile([128, 4, 512], f32, tag="g1")
                for qt in range(4):
                    ps = pp.tile([128, 512], f32, tag="pp")
                    for c in range(4):
                        nc.tensor.matmul(
                            ps[:], xt_sb[:, c, 128 * qt: 128 * (qt + 1)],
                            ws["wg1"][:, c, :],
                            start=(c == 0), stop=(c == 3),
                        )
                    if bg_nz:
                        nc.vector.tensor_add(
                            out=g1_sb[:, qt, :], in0=ps[:], in1=bg_bc[:]
                        )
                    else:
                        nc.scalar.activation(
                            out=g1_sb[:, qt, :], in_=ps[:], func=AF.Copy
                        )
                # HW rejects matmul operands at partition base 64, so build a
                # zero-padded per-head copy of q (other head's 64 rows = 0)
                # and contract over K=128 with the full two-head kT tile.
                q2 = apool.tile([128, 8, CHUNK], bf16, tag="q2")
                for h in range(H):
                    oh = (h % 2) * 64
                    zh = 64 - oh
                    nc.gpsimd.memset(q2[zh: zh + 64, h, :], 0.0)
                    nc.vector.tensor_copy(
                        out=q2[oh: oh + 64, h, :], in_=qT[oh: oh + 64, h // 2, :]
                    )
                # ---- stage 4: banded attention ----
                # (block, half-head-group) granularity: 1-bank PSUM tiles,
                # double-buffered so the PE never waits on exp/mask/normalize
                for b in range(NBLK):
                    ctxn_b = attn_pool.tile([128, 512], bf16, tag="ctxnb")
                    for g in range(2):
                        s0 = sp0.tile([128, 512], f32, tag="s0")
                        s1 = sp1.tile([64, 512], f32, tag="s1")
                        for hh in range(4):
                            h = 4 * g + hh
                            q_ap = q2[:, h, 128 * b: 128 * (b + 1)]
                            nc.tensor.matmul(
                                s0[:, 128 * hh: 128 * (hh + 1)],
                                kT[:, h // 2, 128 * b: 128 * b + 128],
                                q_ap, start=True, stop=True,
                            )
                            nc.tensor.matmul(
                                s1[:, 128 * hh: 128 * (hh + 1)],
                                kT[:, h // 2, 128 * b + 128: 128 * b + 192],
                                q_ap, start=True, stop=True,
                            )
                        a0 = attn_pool.tile([128, 4, 128], bf16, tag="a0")
                        a1 = attn_pool.tile([64, 4, 128], bf16, tag="a1")
                        nc.scalar.activation(
                            out=a0[:].rearrange("p h r -> p (h r)"), in_=s0[:],
                            func=AF.Exp,
                        )
                        nc.scalar.activation(
                            out=a1[:].rearrange("p h r -> p (h r)"), in_=s1[:],
                            func=AF.Exp,
                        )
                        nc.vector.tensor_mul(
                            out=a0[:], in0=a0[:], in1=rep_ap(mA_sb[:, b, :], 1, 4)
                        )
                        nc.vector.tensor_mul(
                            out=a1[:], in0=a1[:], in1=rep_ap(mB_sb[:, b, :], 1, 4)
                        )
                        # AV: ctx_aug[q, 65hh:65hh+65] = A_h @ [1 | V_h]
                        cps = cp.tile([128, 260], f32, tag="cp")
                        for hh in range(4):
                            h = 4 * g + hh
                            col = 65 * hh
                            nc.tensor.matmul(
                                cps[:, col: col + 65], a0[:, hh, :],
                                v_sb[:, b, 65 * h: 65 * (h + 1)],
                                start=True, stop=False,
                            )
                            nc.tensor.matmul(
                                cps[:, col: col + 65], a1[:, hh, :],
                                v_sb[0:64, b + 1, 65 * h: 65 * (h + 1)],
                                start=False, stop=True,
                            )
                        rd = stats_pool.tile([128, 4], f32, tag="rd")
                        nc.vector.reciprocal(
                            out=rd[:],
                            in_=cps[:].rearrange("p (h c) -> p h c", c=65)[:, :, 0],
                        )
                        for hh in range(4):
                            h = 4 * g + hh
                            src_ap = cps[:, 65 * hh + 1: 65 * hh + 65]
                            dst_ap = ctxn_b[:, 64 * h: 64 * (h + 1)]
                            if hh < 2:
                                nc.scalar.activation(
                                    out=dst_ap, in_=src_ap, func=AF.Copy,
                                    scale=rd[:, hh: hh + 1],
                                )
                            else:
                                nc.vector.tensor_scalar_mul(
                                    out=dst_ap, in0=src_ap, scalar1=rd[:, hh: hh + 1],
                                )

                    if stage < 5:
                        continue
                    # ---- per-block epilogue: transpose ctx, O-proj, gate,
                    # blend, store -- pipelines with the next block's attention
                    tp = pp.tile([128, 512], bf16, tag="pp", name=f"tpc{b}")
                    for j in range(4):
                        nc.tensor.transpose(
                            tp[:, 128 * j: 128 * (j + 1)],
                            ctxn_b[:, 128 * j: 128 * (j + 1)],
                            ident[:],
                        )
                    ctxTb = fin_pool.tile([128, 4, 128], bf16, tag="ctxTb")
                    nc.scalar.activation(
                        out=ctxTb[:].rearrange("p c q -> p (c q)"), in_=tp[:],
                        func=AF.Copy,
                    )
                    if stage < 6:
                        continue
                    ops = pp.tile([128, 512], f32, tag="pp")
                    gps = pp.tile([128, 512], f32, tag="pp")
                    for c in range(4):
                        nc.tensor.matmul(
                            ops[:], ctxTb[:, c, :], ws["wo"][:, c, :],
                            start=(c == 0), stop=(c == 3),
                        )
                    for c in range(4):
                        nc.tensor.matmul(
                            gps[:], ctxTb[:, c, :], ws["wg2"][:, c, :],
                            start=(c == 0), stop=(c == 3),
                        )
                    gate = fin_pool.tile([128, 512], f32, tag="gate")
                    diff = fin_pool.tile([128, 512], f32, tag="diff")
                    outs = fin_pool.tile([128, 512], f32, tag="outs")
                    # gate_pre = (x @ Wg1.T, hoisted) + (ctx @ Wg2o.T)
                    nc.vector.tensor_add(out=gps[:], in0=gps[:], in1=g1_sb[:, b, :])
                    nc.scalar.activation(out=gate[:], in_=gps[:], func=AF.Sigmoid)
                    if bo_nz:
                        nc.vector.tensor_add(out=ops[:], in0=ops[:], in1=bo_bc[:])
                    # out = x + gate * (o - x)
                    nc.vector.tensor_sub(out=diff[:], in0=ops[:], in1=x_sb[:, b, :])
                    nc.gpsimd.tensor_mul(out=diff[:], in0=diff[:], in1=gate[:])
                    nc.vector.tensor_add(out=outs[:], in0=diff[:], in1=x_sb[:, b, :])
                    nc.sync.dma_start(
                        out=out_t[:].rearrange("(c p) d -> p c d", p=128)[:, b, :],
                        in_=outs[:],
                    )
            if stage < 6:
                for qt in range(4):
                    nc.sync.dma_start(
                        out=out_t[:].rearrange("(c p) d -> p c d", p=128)[:, qt, :],
                        in_=x_sb[:, qt, :],
                    )
    nc.compile()
    return nc


def _host_prep(inputs):
    """Fold LN gain/bias + scale into weights, build per-core input maps."""
    x = np.asarray(inputs["token_embeds"], np.float32)
    g = np.asarray(inputs["ln_g"], np.float32)
    lb = np.asarray(inputs["ln_b"], np.float32)
    Wp = np.asarray(inputs["Wp"], np.float32)
    Wq = np.asarray(inputs["Wq"], np.float32)
    Wk = np.asarray(inputs["Wk"], np.float32)
    Wv = np.asarray(inputs["Wv"], np.float32)
    Wo = np.asarray(inputs["Wo"], np.float32)
    Wg = np.asarray(inputs["Wg"], np.float32)
    bp = np.asarray(inputs["bp"], np.float32)
    bq = np.asarray(inputs["bq"], np.float32)
    bk = np.asarray(inputs["bk"], np.float32)
    bv = np.asarray(inputs["bv"], np.float32)
    bo = np.asarray(inputs["bo"], np.float32)
    bg = np.asarray(inputs["bg"], np.float32)

    scale = 1.0 / np.sqrt(np.float32(DH))
    wp = np.ascontiguousarray((Wp * g[None, :]).T).astype(BF16)
    wq = np.ascontiguousarray((Wq * scale).T).astype(BF16)
    wk = np.ascontiguousarray((Wk * g[None, :]).T).astype(BF16)
    wv = np.ascontiguousarray((Wv * g[None, :]).T).astype(BF16)
    wo = np.ascontiguousarray(Wo.T).astype(BF16)
    wg1 = np.ascontiguousarray(Wg[:, :D].T).astype(BF16)
    # reference gates on ctx AFTER the O-projection; fold Wo into Wg2 so the
    # gate matmul can consume pre-projection ctx directly
    wg2 = np.ascontiguousarray((Wg[:, D:] @ Wo).T).astype(BF16)
    bp_eff = Wp @ lb + bp
    bq_eff = bq * scale
    bk_eff = Wk @ lb + bk
    bv_eff = (Wv @ lb + bv).reshape(1, D)
    bql = np.ascontiguousarray(bp_eff.reshape(4, 128).T).astype(np.float32)
    bqh = np.ascontiguousarray(bq_eff.reshape(4, 128).T).astype(np.float32)
    bkl = np.ascontiguousarray(bk_eff.reshape(4, 128).T).astype(np.float32)
    bg_eff = Wg[:, D:] @ bo + bg  # gate bias picks up Wg2 @ bo from the fold
    flags = (bool(np.any(bo != 0)), bool(np.any(bg_eff != 0)))

    in_maps = []
    for core in range(NCORES):
        bi, ci = core // 4, core % 4
        s = ci * CHUNK
        xr = np.zeros((XROWS, D), np.float32)
        xr[0:CHUNK] = x[bi, s: s + CHUNK]
        if s - WCTX >= 0:
            xr[CHUNK: CHUNK + WCTX] = x[bi, s - WCTX: s]
        if s + CHUNK + WCTX <= T:
            xr[CHUNK + WCTX: CHUNK + 2 * WCTX] = x[bi, s + CHUNK: s + CHUNK + WCTX]
        xt = np.ascontiguousarray(x[bi, s: s + CHUNK].T).astype(BF16)

        # mask[b, rr, cc]: query r=128b+rr (local), key halo pos j=128b+cc
        rr = np.arange(128)[:, None]
        cc = np.arange(192)[None, :]
        m = np.zeros((NBLK, 128, 192), np.float32)
        for qb in range(NBLK):
            band = (cc - rr >= 0) & (cc - rr <= 2 * WCTX)
            gkey = s + 128 * qb + cc - WCTX + 0 * rr
            m[qb] = (band & (gkey >= 0) & (gkey < T)).astype(np.float32)
        mA = np.ascontiguousarray(m[:, :, :128].transpose(2, 0, 1)).astype(BF16)
        mB = np.ascontiguousarray(m[:, :, 128:].transpose(2, 0, 1)).astype(BF16)

        in_maps.append({
            "x": xr, "xt": xt, "mA": mA, "mB": mB,
            "wp": wp, "wq": wq, "wk": wk, "wv": wv, "wo": wo,
            "wg1": wg1, "wg2": wg2,
            "bql": bql, "bqh": bqh, "bkl": bkl,
            "bv": bv_eff.astype(np.float32),
            "bo": bo.reshape(1, D), "bg": bg_eff.reshape(1, D),
        })
    return in_maps, flags


def _run(inputs, trace=False):
    from concourse.bass_utils import run_bass_kernel_spmd

    in_maps, flags = _host_prep(inputs)
    if flags not in _CACHE:
        _CACHE[flags] = _build_program(flags)
    nc = _CACHE[flags]
    res = run_bass_kernel_spmd(nc, in_maps, list(range(NCORES)), trace=trace)
    out = np.zeros((B, T, D), np.float32)
    for core in range(NCORES):
        bi, ci = core // 4, core % 4
        out[bi, ci * CHUNK: (ci + 1) * CHUNK] = res.results[core]["out"]
    return out, res


def kernel(**inputs):
    out, _ = _run(inputs, trace=False)
    return out



# revision 15
# speedup vs baseline: 1.0259x; 1.0259x over previous
"""Trainium2 Bass kernel for nn_ContextEncoder (banded local attention encoder).

Reference computation (B=2, T=2048, D=512, H=8, dh=64, band half-width 32):
  xn   = LayerNorm(x) * g + b
  q    = ((xn @ Wp.T + bp) @ Wq.T + bq) / sqrt(dh)      per-head [B,T,H,dh]
  k, v = xn @ Wk.T + bk, xn @ Wv.T + bv
  s    = banded scores  (|i-j| <= 32), softmax over window
  ctx  = (a @ v_window) @ Wo.T + bo
  gate = sigmoid([x, ctx] @ Wg.T + bg)
  out  = x * (1 - gate) + ctx * gate

Sharding: sequence-parallel, 8 cores = 2 batches x 4 chunks of 512 tokens.
Each core gets its 512-token chunk plus a 32-token halo on each side
(zero-padded at sequence edges), computes its 512 output rows fully
independently (no collectives), and the host concatenates.

v2 design notes:
  - All inputs packed host-side into contiguous per-partition DRAM lines
    (128 DMA descriptors per tensor); descriptor generation split across the
    Sync and Scalar HWDGE queues.
  - fp8(e4m3) weights (x64 prescale) + fp8 activations for the q/k/v chains;
    DoubleRow matmuls contract 256 elements per instruction.  The gate's
    x-side projection stays bf16 (fp8 there costs ~1e-2 rel err).
  - x ships bf16; the gate's x @ Wg1.T is reconstructed from the normalized
    xn via per-token (mu, sd) LayerNorm stats: x = xn*sd + mu, so
    x@Wg1.T = sd*(xn@Wg1.T) + mu*colsum(Wg1).  No separate x^T upload.
  - Band/boundary masks are pre-seeded into PSUM as 0/-10000 and score
    matmuls accumulate on top (start=False); exp then yields masked probs
    directly, in fp8e5 (scores are O(1) so the range is trivial).
  - AV uses a DoubleRow fp8 matmul per (head, block): subtile 0 = keys
    0..127, subtile 1 = keys 128..191 zero-padded, with a ones-column in V
    producing softmax denominators for free.
  - Per-engine emission is software-pipelined: scores of unit u+1 are
    emitted before AV of unit u so the PE never waits on ACT's exp; the
    gate/blend epilogue is spread across Vector/GpSimd/Scalar.
"""

import os

import numpy as np
import ml_dtypes

KNOB_SEED = os.environ.get("KSEED", "1") == "1"
KNOB_EXP8 = os.environ.get("KEXP8", "1") == "1"
KNOB_DRAV = os.environ.get("KDRAV", "1") == "1"
KNOB_DRPROJ = os.environ.get("KDRPROJ", "1") == "1"

B, T, D = 2, 2048, 512
H, DH = 8, 64
WCTX = 32
NCORES = 8
CHUNK = 512          # tokens per core
NBLK = CHUNK // 128  # 4 query blocks per core
HALO = CHUNK + 2 * WCTX   # 576 tokens incl. halo
BF16 = ml_dtypes.bfloat16
FP8 = ml_dtypes.float8_e4m3
SW = 64.0            # fp8 weight prescale
MASKVAL = -10000.0

_CACHE = {}


def _build_program(flags):
    """Builds the single-core Bass/Tile program (shared SPMD across 8 cores).

    flags: (bq_lo, bq_hi, bk, bv, bo, bg) nonzero-bias booleans.
    """
    import concourse.bass as bass
    import concourse.tile as tile
    import concourse.mybir as mybir
    from concourse import bacc
    from concourse.masks import make_identity

    f32 = mybir.dt.float32
    bf16 = mybir.dt.bfloat16
    fp8e4 = mybir.dt.float8e4
    fp8e5 = mybir.dt.float8e5
    AF = mybir.ActivationFunctionType
    ALU = mybir.AluOpType
    DR = mybir.MatmulPerfMode.DoubleRow
    bq_lo_nz, bq_hi_nz, bk_nz, bv_nz, bo_nz, bg_nz = flags

    nc = bacc.Bacc(
        "TRN2",
        target_bir_lowering=False,
        debug=False,
        enable_asserts=False,
        num_devices=NCORES,
    )

    x_in = nc.dram_tensor("x", [128, 5 * D], bf16, kind="ExternalInput")
    w8_in = {
        n: nc.dram_tensor(n, [128, 4 * D], fp8e4, kind="ExternalInput")
        for n in ["wp", "wq", "wk", "wv"]
    }
    wog_in = nc.dram_tensor("wog", [128, 2 * 4 * D], fp8e4, kind="ExternalInput")
    wg1_in = nc.dram_tensor("wg1", [128, 4 * D], bf16, kind="ExternalInput")
    mseed_in = nc.dram_tensor("mseed", [128, 2 * NBLK * 128], bf16,
                              kind="ExternalInput")
    c1_in = nc.dram_tensor("c1", [1, D], f32, kind="ExternalInput")
    # optional biases, feature-major [128, 4] (col = d-tile) or rows [1, D]
    bql_in = bqh_in = bkl_in = bv_in = bo_in = bg_in = None
    if bq_lo_nz:
        bql_in = nc.dram_tensor("bql", [128, 4], f32, kind="ExternalInput")
    if bq_hi_nz:
        bqh_in = nc.dram_tensor("bqh", [128, 4], f32, kind="ExternalInput")
    if bk_nz:
        bkl_in = nc.dram_tensor("bkl", [128, 4], f32, kind="ExternalInput")
    if bv_nz:
        bv_in = nc.dram_tensor("bv", [1, D], f32, kind="ExternalInput")
    if bo_nz:
        bo_in = nc.dram_tensor("bo", [1, D], f32, kind="ExternalInput")
    if bg_nz:
        bg_in = nc.dram_tensor("bg", [1, D], f32, kind="ExternalInput")
    out_t = nc.dram_tensor("out", [CHUNK, D], f32, kind="ExternalOutput")
    out_v = out_t[:].rearrange("(c p) d -> p c d", p=128)

    def rep_ap(ap, axis_pos, n):
        """Insert a stride-0 dim of size n into an AP at free-dim position."""
        aps = [list(p) for p in ap.ap]
        aps.insert(axis_pos, [0, n])
        return bass.AP(tensor=ap.tensor, offset=ap.offset, ap=aps)

    with tile.TileContext(nc) as tc:
        with (
            tc.tile_pool(name="wpool", bufs=1) as wpool,
            tc.tile_pool(name="apool", bufs=1) as apool,
            tc.tile_pool(name="small", bufs=1) as small,
            tc.tile_pool(name="stats", bufs=3) as stats_pool,
            tc.tile_pool(name="attn", bufs=2) as attn_pool,
            tc.tile_pool(name="fin", bufs=2) as fin_pool,
            tc.tile_pool(name="pp", bufs=2, space="PSUM") as pp,
            tc.tile_pool(name="sp0", bufs=2, space="PSUM") as sp0,
            tc.tile_pool(name="sp1", bufs=2, space="PSUM") as sp1,
            tc.tile_pool(name="cp", bufs=2, space="PSUM") as cp,
        ):
            # ---- persistent SBUF tensors ----
            x_sb = apool.tile([128, 5, D], bf16, tag="x")
            xn0 = apool.tile([128, 5, D], bf16, tag="xn0")
            xnT = apool.tile([128, 4, HALO], bf16, tag="xnT")
            xnT8 = apool.tile([128, 4, HALO], fp8e4, tag="xnT8")
            qinT8 = apool.tile([128, 4, CHUNK], fp8e4, tag="qinT8")
            q2 = apool.tile([128, 8, CHUNK], bf16, tag="q2")
            kT = apool.tile([128, 4, HALO], bf16, tag="kT")
            # v8[p, blk, head, subtile, 65]: col0 = ones (denominator trick)
            v8 = apool.tile([128, NBLK, H, 2, 65], fp8e4, tag="v8")
            # av probs [p, slot, hh, subtile, q]
            av_a = apool.tile([128, 2, 4, 2, 128], fp8e4, tag="av_a")
            g1_sb = apool.tile([128, 4, D], f32, tag="g1")
            musd = apool.tile([128, 5, 2], f32, tag="musd")
            sd = apool.tile([128, 5], f32, tag="sd")
            rinv = apool.tile([128, 5], f32, tag="rinv")

            ws = {}
            for n in w8_in:
                ws[n] = wpool.tile([128, 4, D], fp8e4, tag=n, name=n)
            wog = wpool.tile([128, 2, 4, D], fp8e4, tag="wog")
            wg1 = wpool.tile([128, 4, D], bf16, tag="wg1")
            mseed = wpool.tile([128, 2, NBLK, 128], bf16, tag="mseed")
            c1_bc = wpool.tile([128, D], f32, tag="c1_bc")
            eps_t = small.tile([128, 1], f32, tag="eps")
            ident = small.tile([128, 128], bf16, tag="ident")

            # ---- input DMAs (descriptor gen split across sync/scalar) ----
            nc.sync.dma_start(out=x_sb[:].rearrange("p c d -> p (c d)"),
                              in_=x_in[:])
            nc.scalar.dma_start(out=ws["wp"][:].rearrange("p c d -> p (c d)"),
                                in_=w8_in["wp"][:])
            nc.sync.dma_start(out=ws["wq"][:].rearrange("p c d -> p (c d)"),
                              in_=w8_in["wq"][:])
            nc.scalar.dma_start(out=ws["wk"][:].rearrange("p c d -> p (c d)"),
                                in_=w8_in["wk"][:])
            nc.sync.dma_start(out=ws["wv"][:].rearrange("p c d -> p (c d)"),
                              in_=w8_in["wv"][:])
            nc.scalar.dma_start(out=mseed[:].rearrange("p a b q -> p (a b q)"),
                                in_=mseed_in[:])
            nc.sync.dma_start(out=wg1[:].rearrange("p c d -> p (c d)"),
                              in_=wg1_in[:])
            nc.scalar.dma_start(out=wog[:].rearrange("p w c d -> p (w c d)"),
                                in_=wog_in[:])
            nc.gpsimd.dma_start(out=c1_bc[:], in_=c1_in[:].to_broadcast([128, D]))
            bql = bqh = bkl = bv_bc = bo_bc = bg_bc = None
            if bq_lo_nz:
                bql = small.tile([128, 4], f32, tag="bql")
                nc.sync.dma_start(out=bql[:], in_=bql_in[:])
            if bq_hi_nz:
                bqh = small.tile([128, 4], f32, tag="bqh")
                nc.sync.dma_start(out=bqh[:], in_=bqh_in[:])
            if bk_nz:
                bkl = small.tile([128, 4], f32, tag="bkl")
                nc.sync.dma_start(out=bkl[:], in_=bkl_in[:])
            if bv_nz:
                bv_bc = small.tile([128, D], f32, tag="bv_bc")
                nc.gpsimd.dma_start(out=bv_bc[:], in_=bv_in[:].to_broadcast([128, D]))
            if bo_nz:
                bo_bc = small.tile([128, D], f32, tag="bo_bc")
                nc.gpsimd.dma_start(out=bo_bc[:], in_=bo_in[:].to_broadcast([128, D]))
            if bg_nz:
                bg_bc = small.tile([128, D], f32, tag="bg_bc")
                nc.gpsimd.dma_start(out=bg_bc[:], in_=bg_in[:].to_broadcast([128, D]))

            # ---- init constants (gpsimd, overlaps input DMA) ----
            nc.vector.memset(eps_t[:], 1e-5)
            make_identity(nc, ident[:])
            # zero halves of q2 (per-head dead partitions stay zero forever)
            for h in range(H):
                oh = (h % 2) * 64
                zh = 64 - oh
                nc.gpsimd.memset(q2[zh: zh + 64, h, :], 0.0)
            # av_a subtile-1 rows 64:128 stay zero (63-key segment pad)
            nc.gpsimd.memset(av_a[64:128, :, :, 1, :], 0.0)
            # v8: subtile-1 rows 64:128 zero; "ones" columns hold 2.0 so the
            # denominator reciprocal absorbs the 32->16 ctx rescale for free
            nc.gpsimd.memset(v8[64:128, :, :, 1, :], 0.0)
            nc.gpsimd.memset(v8[:, :, :, 0, 0:1], 2.0)
            nc.gpsimd.memset(v8[0:64, :, :, 1, 0:1], 2.0)

            # ---- stage 1: LayerNorm (token-major) ----
            for t in range(5):
                rows = 128 if t < 4 else 64
                st = stats_pool.tile([128, 6], f32, tag="st")
                nc.vector.bn_stats(out=st[:rows], in_=x_sb[:rows, t, :])
                nc.vector.bn_aggr(out=musd[:rows, t, :], in_=st[:rows])
                nc.scalar.activation(
                    out=sd[:rows, t: t + 1], in_=musd[:rows, t, 1:2],
                    func=AF.Sqrt, bias=eps_t[:rows], scale=1.0,
                )
                nc.vector.reciprocal(out=rinv[:rows, t: t + 1],
                                     in_=sd[:rows, t: t + 1])
                nc.vector.tensor_scalar(
                    out=xn0[:rows, t, :], in0=x_sb[:rows, t, :],
                    scalar1=musd[:rows, t, 0:1], scalar2=rinv[:rows, t: t + 1],
                    op0=ALU.subtract, op1=ALU.mult,
                )

            # ---- stage 2: transpose xn0 -> xnT (PE), cast to fp8 ----
            # x rows: [0:512] central (halo 32..544), [512:544] left halo
            # (halo 0..32), [544:576] right halo (halo 544..576)
            for j in range(4):
                tp = pp.tile([128, 512], bf16, tag="pp", name=f"tpx{j}")
                for t in range(4):
                    nc.tensor.transpose(
                        tp[:, 128 * t: 128 * (t + 1)],
                        xn0[:, t, 128 * j: 128 * (j + 1)],
                        ident[:],
                    )
                if j % 2 == 0:
                    nc.scalar.activation(out=xnT[:, j, 32:544], in_=tp[:],
                                         func=AF.Copy)
                else:
                    nc.vector.tensor_copy(out=xnT[:, j, 32:544], in_=tp[:])
                th = pp.tile([128, 512], bf16, tag="pp", name=f"thx{j}")
                nc.tensor.transpose(
                    th[:, 0:64],
                    xn0[0:64, 4, 128 * j: 128 * (j + 1)],
                    ident[0:64, 0:64],
                )
                _base = xnT[:, j, :]
                halo_out = bass.AP(
                    tensor=_base.tensor, offset=_base.offset,
                    ap=[list(_base.ap[0]), [544, 2], [1, 32]],
                )
                nc.vector.tensor_copy(
                    out=halo_out, in_=th[:, 0:64].rearrange("p (a b) -> p a b", b=32),
                )
                nc.vector.tensor_copy(out=xnT8[:, j, :], in_=xnT[:, j, :])

            # ---- stage 3: projections (fp8 DoubleRow) ----
            # qinT8[d, q] = fp8(2 * (Wp_eff @ xn + bp)); PSUM = 64*qin
            for j in range(4):
                ps = pp.tile([128, 512], f32, tag="pp")
                for c in (0, 2):
                    nc.tensor.matmul(
                        ps[:], ws["wp"][:, c: c + 2, 128 * j: 128 * (j + 1)],
                        xnT8[:, c: c + 2, 32: 32 + CHUNK],
                        start=(c == 0), stop=(c == 2), perf_mode=DR,
                    )
                if bq_lo_nz:
                    nc.scalar.activation(
                        out=qinT8[:, j, :], in_=ps[:], func=AF.Identity,
                        bias=bql[:, j: j + 1], scale=1.0 / 32,
                    )
                else:
                    nc.scalar.activation(
                        out=qinT8[:, j, :], in_=ps[:], func=AF.Copy, scale=1.0 / 32,
                    )
            # q2[d, h, q] = (Wq/8 @ qin + bq/8); PSUM = 64*q
            for j in range(4):
                ps = pp.tile([128, 512], f32, tag="pp")
                for c in (0, 2):
                    nc.tensor.matmul(
                        ps[:], ws["wq"][:, c: c + 2, 128 * j: 128 * (j + 1)],
                        qinT8[:, c: c + 2, :],
                        start=(c == 0), stop=(c == 2), perf_mode=DR,
                    )
                if bq_hi_nz:
                    nc.scalar.activation(
                        out=q2[0:64, 2 * j, :], in_=ps[0:64], func=AF.Identity,
                        bias=bqh[0:64, j: j + 1], scale=1.0 / 64,
                    )
                    nc.vector.tensor_scalar(
                        out=q2[64:128, 2 * j + 1, :], in0=ps[64:128],
                        scalar1=1.0 / 64, scalar2=bqh[64:128, j: j + 1],
                        op0=ALU.mult, op1=ALU.add,
                    )
                else:
                    nc.scalar.activation(out=q2[0:64, 2 * j, :], in_=ps[0:64],
                                         func=AF.Copy, scale=1.0 / 64)
                    nc.vector.tensor_scalar(
                        out=q2[64:128, 2 * j + 1, :], in0=ps[64:128],
                        scalar1=1.0 / 64, scalar2=None, op0=ALU.mult,
                    )
            # kT[d, w] = Wk_eff @ xn + bk (576 halo tokens; central + halo)
            for j in range(4):
                ps = pp.tile([128, 512], f32, tag="pp")
                ph = sp0.tile([128, 512], f32, tag="s0", name=f"kh{j}")
                for c in (0, 2):
                    nc.tensor.matmul(
                        ps[:], ws["wk"][:, c: c + 2, 128 * j: 128 * (j + 1)],
                        xnT8[:, c: c + 2, 0:512],
                        start=(c == 0), stop=(c == 2), perf_mode=DR,
                    )
                for c in (0, 2):
                    nc.tensor.matmul(
                        ph[:, 0:64], ws["wk"][:, c: c + 2, 128 * j: 128 * (j + 1)],
                        xnT8[:, c: c + 2, 512:576],
                        start=(c == 0), stop=(c == 2), perf_mode=DR,
                    )
                if bk_nz:
                    nc.scalar.activation(
                        out=kT[:, j, 0:512], in_=ps[:], func=AF.Identity,
                        bias=bkl[:, j: j + 1], scale=1.0 / 64,
                    )
                    nc.vector.tensor_scalar(
                        out=kT[:, j, 512:576], in0=ph[:, 0:64],
                        scalar1=1.0 / 64, scalar2=bkl[:, j: j + 1],
                        op0=ALU.mult, op1=ALU.add,
                    )
                else:
                    nc.scalar.activation(out=kT[:, j, 0:512], in_=ps[:],
                                         func=AF.Copy, scale=1.0 / 64)
                    nc.vector.tensor_scalar(
                        out=kT[:, j, 512:576], in0=ph[:, 0:64],
                        scalar1=1.0 / 64, scalar2=None, op0=ALU.mult,
                    )
            # v8 = fp8(32 * (v + bv)) token-major, per-head + subtile layout
            for t in range(5):
                rows = 128 if t < 4 else 64
                ps = pp.tile([128, 512], f32, tag="pp")
                for c in (0, 2):
                    nc.tensor.matmul(
                        ps[:rows], xnT8[:, c: c + 2, 128 * t: 128 * t + rows],
                        ws["wv"][:, c: c + 2, :],
                        start=(c == 0), stop=(c == 2), perf_mode=DR,
                    )
                if bv_nz:
                    nc.vector.tensor_add(out=ps[:rows], in0=ps[:rows],
                                         in1=bv_bc[:rows])
                if t < 4:
                    nc.scalar.activation(
                        out=v8[:, t, :, 0, 1:65],
                        in_=ps[:].rearrange("p (h c) -> p h c", c=64),
                        func=AF.Copy, scale=0.5,
                    )
                if t >= 1:
                    nc.vector.tensor_scalar(
                        out=v8[0:64, t - 1, :, 1, 1:65],
                        in0=ps[0:64].rearrange("p (h c) -> p h c", c=64),
                        scalar1=0.5, scalar2=None, op0=ALU.mult,
                    )

            # ---- attention + epilogue, software-pipelined ----
            NU = 2 * NBLK  # units: (block, half-head-group)

            def emit_g1(b):
                # g1_sb = 1024*(x @ Wg1.T) = sd*(xn@wg1) + mu*c1
                ps = pp.tile([128, 512], f32, tag="pp", name=f"g1p{b}")
                for c in range(4):
                    nc.tensor.matmul(
                        ps[:], xnT[:, c, 32 + 128 * b: 32 + 128 * (b + 1)],
                        wg1[:, c, :],
                        start=(c == 0), stop=(c == 3),
                    )
                tmp = stats_pool.tile([128, 512], f32, tag="g1tmp")
                nc.vector.tensor_scalar(
                    out=tmp[:], in0=c1_bc[:], scalar1=musd[:, b, 0:1],
                    scalar2=None, op0=ALU.mult,
                )
                nc.vector.scalar_tensor_tensor(
                    out=g1_sb[:, b, :], in0=ps[:], scalar=sd[:, b: b + 1],
                    in1=tmp[:], op0=ALU.mult, op1=ALU.add,
                )
                if bg_nz:
                    nc.vector.tensor_add(out=g1_sb[:, b, :], in0=g1_sb[:, b, :],
                                         in1=bg_bc[:])

            units = {}

            def emit_front(u):
                b, g = u // 2, u % 2
                s0 = sp0.tile([128, 512], f32, tag="s0")
                s1 = sp1.tile([64, 512], f32, tag="s1")
                # seed scores PSUM with the band/boundary mask (0 / -10000)
                if KNOB_SEED:
                    nc.vector.tensor_copy(
                        out=s0[:].rearrange("p (a q) -> p a q", q=128),
                        in_=rep_ap(mseed[:, 0, b, :], 1, 4),
                    )
                    nc.vector.tensor_copy(
                        out=s1[:].rearrange("p (a q) -> p a q", q=128),
                        in_=rep_ap(mseed[0:64, 1, b, :], 1, 4),
                    )
                # scores: S^T[key, (pair, head, q)] accumulated onto the seed
                for p in range(2):
                    j = 2 * g + p
                    nc.tensor.matmul(
                        s0[:, 256 * p: 256 * (p + 1)],
                        kT[:, j, 128 * b: 128 * b + 128],
                        q2[:, 2 * j: 2 * j + 2, 128 * b: 128 * (b + 1)],
                        start=not KNOB_SEED, stop=True, skip_group_check=True,
                    )
                    nc.tensor.matmul(
                        s1[:, 256 * p: 256 * (p + 1)],
                        kT[:, j, 128 * b + 128: 128 * b + 192],
                        q2[:, 2 * j: 2 * j + 2, 128 * b: 128 * (b + 1)],
                        start=not KNOB_SEED, stop=True, skip_group_check=True,
                    )
                units[u] = (s0, s1)

            def emit_back(u, ctxn_b):
                b, g = u // 2, u % 2
                s0, s1 = units.pop(u)
                slot = u % 2
                # exp -> masked probs in fp8 (unnormalized)
                if KNOB_EXP8:
                    nc.scalar.activation(
                        out=av_a[:, slot, :, 0, :],
                        in_=s0[:].rearrange("p (a q) -> p a q", q=128),
                        func=AF.Exp,
                    )
                    nc.scalar.activation(
                        out=av_a[0:64, slot, :, 1, :],
                        in_=s1[:].rearrange("p (a q) -> p a q", q=128),
                        func=AF.Exp,
                    )
                else:
                    avb = attn_pool.tile([128, 4, 2, 128], bf16, tag="avb")
                    nc.scalar.activation(
                        out=avb[:, :, 0, :],
                        in_=s0[:].rearrange("p (a q) -> p a q", q=128),
                        func=AF.Exp,
                    )
                    nc.scalar.activation(
                        out=avb[0:64, :, 1, :],
                        in_=s1[:].rearrange("p (a q) -> p a q", q=128),
                        func=AF.Exp,
                    )
                    nc.vector.tensor_copy(out=av_a[:, slot, :, 0, :],
                                          in_=avb[:, :, 0, :])
                    nc.vector.tensor_copy(out=av_a[0:64, slot, :, 1, :],
                                          in_=avb[0:64, :, 1, :])
                # AV: ctx_aug[q, 65*hh : 65*hh+65] = A_h @ [1 | V_h]
                cps = cp.tile([128, 260], f32, tag="cp")
                for hh in range(4):
                    h = 4 * g + hh
                    if KNOB_DRAV:
                        nc.tensor.matmul(
                            cps[:, 65 * hh: 65 * hh + 65],
                            av_a[:, slot, hh, :, :],
                            v8[:, b, h, :, :],
                            start=True, stop=True, perf_mode=DR,
                        )
                    else:
                        nc.tensor.matmul(
                            cps[:, 65 * hh: 65 * hh + 65],
                            av_a[:, slot, hh, 0, :],
                            v8[:, b, h, 0, :],
                            start=True, stop=False,
                        )
                        nc.tensor.matmul(
                            cps[:, 65 * hh: 65 * hh + 65],
                            av_a[0:64, slot, hh, 1, :],
                            v8[0:64, b, h, 1, :],
                            start=False, stop=True,
                        )
                rd = stats_pool.tile([128, 4], f32, tag="rd")
                nc.vector.reciprocal(
                    out=rd[:],
                    in_=cps[:].rearrange("p (h c) -> p h c", c=65)[:, :, 0],
                )
                # ctxn = 16*ctx bf16 (cps = 32*unnorm; the 2.0 ones-column
                # already folded the 0.5 into rd)
                for hh in range(4):
                    h = 4 * g + hh
                    src = cps[:, 65 * hh + 1: 65 * hh + 65]
                    dst = ctxn_b[:, 64 * h: 64 * (h + 1)]
                    if hh < 2:
                        nc.vector.tensor_scalar(
                            out=dst, in0=src, scalar1=rd[:, hh: hh + 1],
                            scalar2=None, op0=ALU.mult,
                        )
                    else:
                        nc.scalar.activation(
                            out=dst, in_=src, func=AF.Copy,
                            scale=rd[:, hh: hh + 1],
                        )

            def emit_epi_t(b, ctxn_b, fins):
                # transpose ctx to feature-major, cast to fp8
                tp = pp.tile([128, 512], bf16, tag="pp", name=f"tpc{b}")
                for j in range(4):
                    nc.tensor.transpose(
                        tp[:, 128 * j: 128 * (j + 1)],
                        ctxn_b[:, 128 * j: 128 * (j + 1)],
                        ident[:],
                    )
                ctxT = fin_pool.tile([128, 4, 128], bf16, tag="ctxT")
                nc.scalar.activation(
                    out=ctxT[:].rearrange("p c q -> p (c q)"), in_=tp[:],
                    func=AF.Copy,
                )
                ctxT8 = fin_pool.tile([128, 4, 128], fp8e4, tag="ctxT8")
                nc.vector.tensor_copy(out=ctxT8[:], in_=ctxT[:])
                fins[b] = ctxT8

            def emit_epi_p(b, fins):
                ctxT8 = fins.pop(b)
                ops = pp.tile([128, 512], f32, tag="pp", name=f"op{b}")
                gps = pp.tile([128, 512], f32, tag="pp", name=f"gp{b}")
                for c in (0, 2):
                    nc.tensor.matmul(
                        ops[:], ctxT8[:, c: c + 2, :], wog[:, 0, c: c + 2, :],
                        start=(c == 0), stop=(c == 2), perf_mode=DR,
                    )
                for c in (0, 2):
                    nc.tensor.matmul(
                        gps[:], ctxT8[:, c: c + 2, :], wog[:, 1, c: c + 2, :],
                        start=(c == 0), stop=(c == 2), perf_mode=DR,
                    )
                gsum = fin_pool.tile([128, 512], f32, tag="gsum")
                gate = fin_pool.tile([128, 512], f32, tag="gate")
                diff = fin_pool.tile([128, 512], f32, tag="diff")
                outs = fin_pool.tile([128, 512], f32, tag="outs")
                # gate_pre*1024 = gps + g1_sb; o*1024 = ops
                nc.vector.tensor_add(out=gsum[:], in0=gps[:], in1=g1_sb[:, b, :])
                nc.scalar.activation(out=gate[:], in_=gsum[:], func=AF.Sigmoid,
                                     scale=1.0 / 1024)
                # out = x + gate * (o - x)
                nc.vector.scalar_tensor_tensor(
                    out=diff[:], in0=ops[:], scalar=1.0 / 1024,
                    in1=x_sb[:, b, :], op0=ALU.mult, op1=ALU.subtract,
                )
                if bo_nz:
                    nc.vector.tensor_add(out=diff[:], in0=diff[:], in1=bo_bc[:])
                nc.gpsimd.tensor_mul(out=diff[:], in0=diff[:], in1=gate[:])
                nc.vector.tensor_add(out=outs[:], in0=diff[:], in1=x_sb[:, b, :])
                eng = nc.sync if b % 2 == 0 else nc.scalar
                eng.dma_start(out=out_v[:, b, :], in_=outs[:])

            # pipeline: scores(u+1) ahead of AV(u); g1 blocks + epilogues as
            # PE filler between units so the PE never waits on ACT's exp
            ctxns = {}
            fins = {}
            for bb in range(NBLK):
                ctxns[bb] = attn_pool.tile([128, 512], bf16, tag="ctxnb",
                                           name=f"ctxn{bb}")
            emit_front(0)
            emit_front(1)
            emit_g1(0)
            emit_back(0, ctxns[0])
            emit_front(2)
            emit_g1(1)
            emit_back(1, ctxns[0])
            emit_front(3)
            emit_g1(2)
            emit_back(2, ctxns[1])
            emit_epi_t(0, ctxns[0], fins)
            emit_front(4)
            emit_g1(3)
            emit_back(3, ctxns[1])
            emit_epi_p(0, fins)
            emit_front(5)
            emit_back(4, ctxns[2])
            emit_epi_t(1, ctxns[1], fins)
            emit_front(6)
            emit_back(5, ctxns[2])
            emit_epi_p(1, fins)
            emit_front(7)
            emit_back(6, ctxns[3])
            emit_epi_t(2, ctxns[2], fins)
            emit_back(7, ctxns[3])
            emit_epi_p(2, fins)
            emit_epi_t(3, ctxns[3], fins)
            emit_epi_p(3, fins)
    nc.compile()
    return nc


def _host_prep(inputs):
    """Fold LN gain/bias + scales into weights, build per-core input maps."""
    x = np.asarray(inputs["token_embeds"], np.float32)
    g = np.asarray(inputs["ln_g"], np.float32)
    lb = np.asarray(inputs["ln_b"], np.float32)
    Wp = np.asarray(inputs["Wp"], np.float32)
    Wq = np.asarray(inputs["Wq"], np.float32)
    Wk = np.asarray(inputs["Wk"], np.float32)
    Wv = np.asarray(inputs["Wv"], np.float32)
    Wo = np.asarray(inputs["Wo"], np.float32)
    Wg = np.asarray(inputs["Wg"], np.float32)
    bp = np.asarray(inputs["bp"], np.float32)
    bq = np.asarray(inputs["bq"], np.float32)
    bk = np.asarray(inputs["bk"], np.float32)
    bv = np.asarray(inputs["bv"], np.float32)
    bo = np.asarray(inputs["bo"], np.float32)
    bg = np.asarray(inputs["bg"], np.float32)

    scale = 1.0 / np.sqrt(np.float32(DH))

    def pack8(w, s):
        # [D, D] -> fp8(s * w) packed [128, 4*D] with row 128c+p at (p, c)
        q = (w * s).astype(FP8)
        return np.ascontiguousarray(
            q.reshape(4, 128, D).transpose(1, 0, 2).reshape(128, 4 * D))

    def packb(w, s):
        q = (w * s).astype(BF16)
        return np.ascontiguousarray(
            q.reshape(4, 128, D).transpose(1, 0, 2).reshape(128, 4 * D))

    wp8 = pack8(np.ascontiguousarray((Wp * g[None, :]).T), SW)
    wq8 = pack8(np.ascontiguousarray((Wq * scale).T), SW / 2)
    wk8 = pack8(np.ascontiguousarray((Wk * g[None, :]).T), SW)
    wv8 = pack8(np.ascontiguousarray((Wv * g[None, :]).T), SW)
    wo8 = pack8(np.ascontiguousarray(Wo.T), SW)
    W2o = Wg[:, D:] @ Wo
    wg28 = pack8(np.ascontiguousarray(W2o.T), SW)
    wog = np.ascontiguousarray(np.concatenate([wo8, wg28], axis=1))
    wg1 = packb(np.ascontiguousarray(Wg[:, :D].T), 1024.0)
    c1 = (1024.0 * Wg[:, :D].sum(axis=1)).reshape(1, D).astype(np.float32)

    bp_eff = (Wp @ lb + bp) * 2.0       # qin stored as 2*qin
    bq_eff = bq * scale
    bk_eff = Wk @ lb + bk
    bv_eff = (Wv @ lb + bv).reshape(1, D)
    bg_eff = (Wg[:, D:] @ bo + bg) * 1024.0
    bql = np.ascontiguousarray(bp_eff.reshape(4, 128).T).astype(np.float32)
    bqh = np.ascontiguousarray(bq_eff.reshape(4, 128).T).astype(np.float32)
    bkl = np.ascontiguousarray(bk_eff.reshape(4, 128).T).astype(np.float32)
    flags = (
        bool(np.any(bql != 0)), bool(np.any(bqh != 0)), bool(np.any(bkl != 0)),
        bool(np.any(bv_eff != 0)), bool(np.any(bo != 0)), bool(np.any(bg_eff != 0)),
    )

    in_maps = []
    for core in range(NCORES):
        bi, ci = core // 4, core % 4
        s = ci * CHUNK
        # x rows: 512 central + 32 left halo + 32 right halo + pad to 640
        xr = np.zeros((5 * 128, D), np.float32)
        xr[0:CHUNK] = x[bi, s: s + CHUNK]
        if s - WCTX >= 0:
            xr[CHUNK: CHUNK + WCTX] = x[bi, s - WCTX: s]
        if s + CHUNK + WCTX <= T:
            xr[CHUNK + WCTX: CHUNK + 2 * WCTX] = x[bi, s + CHUNK: s + CHUNK + WCTX]
        xb = np.ascontiguousarray(
            xr.astype(BF16).reshape(5, 128, D).transpose(1, 0, 2).reshape(128, 5 * D))

        # mask seeds [k, block, q]: 0 valid, MASKVAL invalid
        rr = np.arange(128)[:, None]
        cc = np.arange(192)[None, :]
        m = np.full((NBLK, 128, 192), MASKVAL, np.float32)
        for qb in range(NBLK):
            band = (cc - rr >= 0) & (cc - rr <= 2 * WCTX)
            gkey = s + 128 * qb + cc - WCTX + 0 * rr
            m[qb][band & (gkey >= 0) & (gkey < T)] = 0.0
        mA = m[:, :, :128].transpose(2, 0, 1)      # [128 k, NBLK, 128 q]
        mBp = np.full((128, NBLK, 128), MASKVAL, np.float32)
        mBp[0:64] = m[:, :, 128:].transpose(2, 0, 1)[0:64]
        mseed = np.ascontiguousarray(
            np.stack([mA, mBp], axis=1).astype(BF16).reshape(128, 2 * NBLK * 128))

        im = {
            "x": xb, "mseed": mseed, "c1": c1,
            "wp": wp8, "wq": wq8, "wk": wk8, "wv": wv8,
            "wog": wog, "wg1": wg1,
        }
        if flags[0]:
            im["bql"] = bql
        if flags[1]:
            im["bqh"] = bqh
        if flags[2]:
            im["bkl"] = bkl
        if flags[3]:
            im["bv"] = (bv_eff * SW).astype(np.float32)  # added to PSUM = 64*v
        if flags[4]:
            im["bo"] = bo.reshape(1, D).astype(np.float32)
        if flags[5]:
            im["bg"] = bg_eff.reshape(1, D).astype(np.float32)
        in_maps.append(im)
    return in_maps, flags


def _run(inputs, trace=False):
    from concourse.bass_utils import run_bass_kernel_spmd

    in_maps, flags = _host_prep(inputs)
    if flags not in _CACHE:
        _CACHE[flags] = _build_program(flags)
    nc = _CACHE[flags]
    res = run_bass_kernel_spmd(nc, in_maps, list(range(NCORES)), trace=trace)
    out = np.zeros((B, T, D), np.float32)
    for core in range(NCORES):
        bi, ci = core // 4, core % 4
        out[bi, ci * CHUNK: (ci + 1) * CHUNK] = res.results[core]["out"]
    return out, res


def kernel(**inputs):
    out, _ = _run(inputs, trace=False)
    return out
